# revision 40
# baseline (speedup 1.0000x reference)
"""Trainium2 Bass kernel for a binarized (1w1a) BasicBlock:

    out = relu(bn2(conv2(sign(pad(relu(bn1(conv1(sign(pad(x)), sign(w1)))))), sign(w2))) + x)

with 2x3 convs, C=256, B=64, H=W=32, pad = (W: 1 left/right, H: 1 bottom).

Strategy: data-parallel over batch across 8 NeuronCores (8 images/core).
Per core each conv is an implicit GEMM: input channels on the 128 SBUF
partitions, contraction over all 256 channels in a single PE pass via fp8e4
DoubleRow matmuls (binarized values +-1/0 are exact in fp8; PSUM accumulates
fp32, so all conv sums are exact integers). Activations live in a
"shared-pad" plane layout: 33-wide rows where one zero column serves as both
the right pad of row h and the left pad of row h+1, so each of the 6 kernel
taps is a contiguous 363-column slice per 11-row chunk and the conv reduces
to 6 PSUM-accumulated matmuls per chunk.

Host-side prep: weights are binarized and laid out as DoubleRow lhsT tiles;
BN is folded into per-channel scale/bias; conv1's input is binarized and
packed into the padded fp8 planes on the host, so the device's first matmul
only waits on a ~150KB DMA. conv1's bn+relu+sign epilogue collapses into one
DVE tensor_scalar ((psum*inv1) is_gt (-bias1) -> {0,1}) writing straight
into conv2's input planes; conv2's epilogue is scalar_tensor_tensor
(psum*inv2 + x) followed by a per-channel-bias Relu. A short dummy-matmul
warm-up keeps the PE's HAM clock-gate at full rate from the first real
matmul, and DMA traffic is spread across the sync/scalar HWDGE queues (plus
gpsimd SWDGE for non-critical transfers).
"""

import numpy as np
import ml_dtypes

import concourse.mybir as mybir
import concourse.tile as tile
from concourse import bacc
from concourse.bass_utils import run_bass_kernel_spmd

N_CORES = 8
B, C, H, W = 64, 256, 32, 32
BL = B // N_CORES          # images per core
P = 128
KT = C // P                # channel tiles (contraction / output)
HP, WP = H + 1, W + 2      # padded spatial dims (33, 34)
IMG = HP * WP              # 1122
NPOS = 6                   # 2x3 kernel taps
EPS = 1e-5

F32 = mybir.dt.float32
BF16 = mybir.dt.bfloat16
FP8 = mybir.dt.float8e4

# fp8 DoubleRow variant: shared-pad plane layout. Each padded row is 33 wide
# (32 data + 1 shared zero column that serves as row h's right pad AND row
# h+1's left pad), plus one leading zero and a zero bottom row. Conv output
# (h, w) lands at flat position h*33 + w of the 363-column PSUM chunks.
PITCH = 33
DATA0 = 1                   # leading zero (left pad of row 0)
PLANE = DATA0 + PITCH * PITCH   # 1090 = data extent incl bottom pad row
NJ = 3                      # chunks per image (11 rows each)
CH = 11 * PITCH             # 363
NPAD = 1168                 # >= 2*CH + max tap offset (67) + CH, mult of 16
ROWS_J = (11, 11, 10)       # valid output rows per chunk

VARIANT = "fp8"             # "bf16" | "fp8"

_CACHE = {}


def _build():
    return _build_fp8()


def _build_fp8():
    """fp8e4 DoubleRow kernel.

    conv1's binarized input arrives from the host already packed in the
    shared-pad fp8 plane layout (xq1), so the first matmul only waits for a
    ~150KB DMA. The f32 x (residual) and weights stream in behind it. Both
    convs are 6-tap DoubleRow PSUM accumulations over 363-column chunks;
    conv1's epilogue binarizes straight into conv2's input planes.
    """
    if "nc" in _CACHE:
        return _CACHE["nc"]

    nc = bacc.Bacc("TRN2", target_bir_lowering=False, debug=False)

    xq_d = nc.dram_tensor("xq1", [P, BL, KT, NPAD], FP8, kind="ExternalInput")
    x_d = nc.dram_tensor("x", [P, BL, KT, H * W], F32, kind="ExternalInput")
    w1_d = nc.dram_tensor("w1t", [KT, P, KT, NPOS, P], FP8, kind="ExternalInput")
    w2_d = nc.dram_tensor("w2t", [KT, P, KT, NPOS, P], FP8, kind="ExternalInput")
    bnv_d = nc.dram_tensor("bnv", [4, C], F32, kind="ExternalInput")
    out_d = nc.dram_tensor("out", [BL, C, H, W], F32, kind="ExternalOutput")

    with tile.TileContext(nc) as tc:
        with (
            tc.tile_pool(name="res", bufs=1) as res,
            tc.tile_pool(name="tmp", bufs=4) as tmp,
            tc.tile_pool(name="stg", bufs=4) as stg,
            tc.tile_pool(name="ps", bufs=6, space="PSUM") as ps,
        ):
            xq1 = [None] * BL
            xq2 = [None] * BL
            xg = [None] * BL

            def pad_memsets(q, eng):
                """Zero the pad cells of a fresh plane tile: leading zero,
                shared pad column, bottom pad row, tail."""
                v = q[:, :, DATA0:DATA0 + PITCH * PITCH].rearrange(
                    "c k (h w) -> c k h w", w=PITCH)
                eng.memset(q[:, :, 0:DATA0], 0.0)
                eng.memset(v[:, :, :, W:PITCH], 0.0)
                eng.memset(v[:, :, H:PITCH, 0:W], 0.0)
                eng.memset(q[:, :, PLANE:NPAD], 0.0)

            def interior(q, kt):
                return q[:, kt, DATA0:DATA0 + H * PITCH].rearrange(
                    "c (h w) -> c h w", w=PITCH)[:, :, 0:W]

            # PE warm-up while inputs land: HAM reaches 8/8 before the first
            # real matmul.
            wu = res.tile([P, 512], FP8, tag="wu", name="wu")
            nc.vector.memset(wu[:], 0.0)
            wups = ps.tile([P, 512], F32, tag="wups", name="wups", bufs=1)
            for _ in range(4):
                nc.tensor.matmul(wups[:], wu[:, 0:P], wu[:], start=True, stop=True)

            # conv1 inputs: host-packed planes, one small DMA per image on
            # the sync queue (b0 first -- it gates the first matmul group)
            for b in range(BL):
                xq1[b] = res.tile([P, KT, NPAD], FP8, tag=f"xq1_{b}", name=f"xq1_{b}")
                nc.sync.dma_start(xq1[b][:], xq_d.ap()[:, b])

            # weights (mt=0 half first) + BN on the scalar queue
            w1sb, w2sb = [None, None], [None, None]
            for mt in range(KT):
                w1sb[mt] = res.tile([P, KT, NPOS, P], FP8, tag=f"w1q{mt}", name=f"w1q{mt}")
                nc.scalar.dma_start(w1sb[mt][:], w1_d.ap()[mt])
            bnsb = res.tile([P, 4 * KT], F32, tag="bnv", name="bnv")
            nc.scalar.dma_start(bnsb[:], bnv_d.ap().rearrange("v (t p) -> p (v t)", p=P))
            for mt in range(KT):
                w2sb[mt] = res.tile([P, KT, NPOS, P], FP8, tag=f"w2q{mt}", name=f"w2q{mt}")
                nc.scalar.dma_start(w2sb[mt][:], w2_d.ap()[mt])

            inv1sb = bnsb[:, 0 * KT:1 * KT]
            nb1sb = bnsb[:, 1 * KT:2 * KT]
            inv2sb = bnsb[:, 2 * KT:3 * KT]
            b2sb = bnsb[:, 3 * KT:4 * KT]

            # conv2 input planes: zero the pads (vector/gpsimd, never scalar)
            for b in range(BL):
                xq2[b] = res.tile([P, KT, NPAD], FP8, tag=f"xq2_{b}", name=f"xq2_{b}")
                pad_memsets(xq2[b], nc.vector if b % 2 else nc.gpsimd)

            # residual x (f32): needed only by conv2's epilogue; stream in
            # behind everything else, alternating the two HW queues
            for b in range(BL):
                xg[b] = res.tile([P, KT, H * W], F32, tag=f"xg{b}", name=f"xg{b}")
                (nc.scalar if b % 2 else nc.sync).dma_start(xg[b][:], x_d.ap()[:, b])

            def conv_groups(b, mt, wsb, src):
                """6-tap DoubleRow accumulation for the NJ chunks of (b, mt);
                pos-outer / chunk-inner so consecutive matmuls share lhsT."""
                pts = [
                    ps.tile([P, CH], F32, tag="ps", name=f"ps_{b}_{mt}_{j}")
                    for j in range(NJ)
                ]
                for pos in range(NPOS):
                    kh, kw = divmod(pos, 3)
                    off = kh * PITCH + kw
                    for j in range(NJ):
                        nc.tensor.matmul(
                            pts[j][:],
                            wsb[mt][:, :, pos, :],
                            src[:, :, off + j * CH: off + j * CH + CH],
                            start=(pos == 0),
                            stop=(pos == NPOS - 1),
                            perf_mode=mybir.MatmulPerfMode.DoubleRow,
                        )
                return pts

            # ---- conv1 + binarize epilogue ----
            for b in range(BL):
                for mt in range(KT):
                    pts = conv_groups(b, mt, w1sb, xq1[b])
                    q2v = interior(xq2[b], mt)
                    for j in range(NJ):
                        r = ROWS_J[j]
                        nc.vector.tensor_scalar(
                            q2v[:, 11 * j:11 * j + r, :],
                            pts[j].rearrange("c (r w) -> c r w", w=PITCH)[:, 0:r, 0:W],
                            inv1sb[:, mt:mt + 1],
                            nb1sb[:, mt:mt + 1],
                            mybir.AluOpType.mult,
                            mybir.AluOpType.is_gt,
                        )

            # ---- conv2 + bn2 + residual + relu ----
            OUTQ = (nc.sync, nc.gpsimd, nc.scalar)
            for b in range(BL):
                for mt in range(KT):
                    pts = conv_groups(b, mt, w2sb, xq2[b])
                    ot = stg.tile([P, H * W], F32, tag="ot", name=f"ot_{b}_{mt}")
                    for j in range(NJ):
                        r = ROWS_J[j]
                        n = r * W
                        n0 = 11 * j * W
                        tt = tmp.tile([P, 11 * W], F32, tag="t2", name=f"t2_{b}_{mt}_{j}")
                        nc.vector.scalar_tensor_tensor(
                            tt[:, 0:n].rearrange("c (r w) -> c r w", w=W),
                            pts[j].rearrange("c (r w) -> c r w", w=PITCH)[:, 0:r, 0:W],
                            inv2sb[:, mt:mt + 1],
                            xg[b][:, mt, n0:n0 + n].rearrange("c (r w) -> c r w", w=W),
                            mybir.AluOpType.mult,
                            mybir.AluOpType.add,
                        )
                        nc.scalar.activation(
                            ot[:, n0:n0 + n], tt[:, 0:n],
                            mybir.ActivationFunctionType.Relu,
                            bias=b2sb[:, mt:mt + 1],
                            scale=1.0,
                        )
                        if b == BL - 1:
                            # both HW queues are idle at the tail; avoid SWDGE
                            (nc.sync if j != 1 else nc.scalar).dma_start(
                                out_d.ap()[b, mt * P:(mt + 1) * P]
                                     .rearrange("c h w -> c (h w)")[:, n0:n0 + n],
                                ot[:, n0:n0 + n],
                            )
                    if b < BL - 1:
                        OUTQ[(b * KT + mt) % 3].dma_start(
                            out_d.ap()[b, mt * P:(mt + 1) * P].rearrange("c h w -> c (h w)"),
                            ot[:],
                        )

    nc.compile()
    _CACHE["nc"] = nc
    return nc


def _prep(w1, w2, gamma1, beta1, mean1, var1, gamma2, beta2, mean2, var2):
    """Host-side: fold BN, binarize + lay out weights as lhsT tiles."""
    def fold(gamma, beta, mean, var):
        inv = (gamma.astype(np.float64) / np.sqrt(var.astype(np.float64) + EPS))
        inv = inv.astype(np.float32)
        bias = (beta.astype(np.float32) - mean.astype(np.float32) * inv)
        return inv, bias

    inv1, bias1 = fold(gamma1, beta1, mean1, var1)
    inv2, bias2 = fold(gamma2, beta2, mean2, var2)

    def wt(w):
        # [O, I, 2, 3] -> DoubleRow lhsT layout [mt, ci, ko, pos, co']
        s = np.sign(w).astype(np.float32)
        arr = s.transpose(1, 2, 3, 0).reshape(KT, P, NPOS, KT, P)  # [ko,ci,pos,mt,co']
        arr = arr.transpose(3, 1, 0, 2, 4)
        return np.ascontiguousarray(arr).astype(mybir.dt.np(FP8))

    bnv = np.ascontiguousarray(np.stack([inv1, -bias1, inv2, bias2]))
    return wt(w1), wt(w2), bnv


# flat positions of the plane interior (row h, col c) -> DATA0 + h*PITCH + c
_INT_COLS = (DATA0 + (np.arange(H)[:, None] * PITCH + np.arange(W))).ravel()


def _in_maps(x, w1t, w2t, bnv):
    """Per-core input dicts: xq1 = sign(x) packed into the shared-pad fp8
    plane layout [p, b, kt, NPAD]; x = f32 residual in [p, b, kt, hw]."""
    maps = []
    for c in range(N_CORES):
        xs = x[c * BL:(c + 1) * BL]                       # [BL, C, H, W]
        xh = np.ascontiguousarray(
            xs.reshape(BL, KT, P, H * W).transpose(2, 0, 1, 3))
        v = np.sign(xh)                                   # [P, BL, KT, H*W]
        plane = np.zeros((P, BL, KT, NPAD), np.float32)
        plane[:, :, :, _INT_COLS] = v
        xq = plane.astype(mybir.dt.np(FP8))
        maps.append({"xq1": xq, "x": xh, "w1t": w1t, "w2t": w2t, "bnv": bnv})
    return maps


def kernel(x, w1, gamma1, beta1, mean1, var1,
           w2, gamma2, beta2, mean2, var2):
    x = np.asarray(x, dtype=np.float32)
    w1t, w2t, bnv = _prep(
        np.asarray(w1), np.asarray(w2),
        np.asarray(gamma1), np.asarray(beta1), np.asarray(mean1), np.asarray(var1),
        np.asarray(gamma2), np.asarray(beta2), np.asarray(mean2), np.asarray(var2),
    )

    nc = _build()
    in_maps = _in_maps(x, w1t, w2t, bnv)

    res = run_bass_kernel_spmd(nc, in_maps, core_ids=list(range(N_CORES)))
    out = np.concatenate([r["out"] for r in res.results], axis=0)
    return out
